# revision 62
# baseline (speedup 1.0000x reference)
"""Multi-head attention kernel for Trainium2, SPMD over 8 NeuronCores.

Problem: B=4, N=2048, C=1024, 16 heads, head_dim=64 (fp32 reference).

Sharding: core = (batch b, head-group hg) with b = core//2, hg = core%2.
Each core computes attention for its 8 heads of its batch and a PARTIAL
projection output [N, C]; the host sums the two partials per batch and adds
the bias. No on-chip collectives needed.

Per-core dataflow (all matmul inputs bf16, f32 PSUM accumulation):
  qkT[o,n]  = w_qk_local @ x^T          (o = [q heads | k heads], 1024 rows)
  v[n,dl]   = x @ w_v_local^T           (natural layout, augmented with ones col)
  S_T[k,q]  = kT_h^T q (per head)       -> 2^(S_T) on ScalarE -> E bf16
              (q pre-scaled by 0.125*log2(e) on host, ACT scale=ln2)
  numT[d,q] = v_aug^T E  (M=65: row 64 = sumexp)   accumulated over k chunks
  outT[d,q] = numT[0:64] * (1/sumexp)   (recip via DMA gather + DVE, DMA bcast)
  partial[n,o] = outT^T @ w_proj_local^T chunks    -> DMA out (f32)

Scheduling (the perf-critical part; measured on HW):
  * Consecutive full-array (auto-tiled) matmuls chain at ~226 ns (their
    LDWEIGHTS hides via the background weight buffer); explicitly tiled
    matmuls pay ~+90 ns per category transition.  So AV stays M=65 (full
    tiling) and the only tiled matmuls are the QK row-pair (which runs
    internally concurrent at +4 ns).
  * The chunk loop is software-pipelined: QK(i+1) is emitted BEFORE AV(i),
    so the PE FIFO never idles while ScalarE computes exp(i) - the per-chunk
    serial QK->exp->AV chain was the baseline's main stall.
  * QKV projection (phase B) and the output projection run as "filler"
    matmuls pumped into the PE slack of the exp-gated chunk stream.
"""
import os
import sys
import types
import time
import numpy as np
import ml_dtypes
from contextlib import ExitStack

import concourse.bass as bass
import concourse.tile as tile
from concourse import bacc, mybir

BF16 = mybir.dt.bfloat16
F32 = mybir.dt.float32
I32 = mybir.dt.int32

N = 2048          # sequence length
C = 1024          # model dim
HL = 8            # heads per core
D = 64            # head dim
SCALE = D ** -0.5
NCORES = 8
LN2 = float(np.log(2.0))
LOG2E = float(np.log2(np.e))

KC = N // 128     # 16 k-chunks per head
QB = 4            # q blocks of 512
QBS = N // QB     # 512
CCH = C // 128    # 8 contraction chunks


# ---------------------------------------------------------------------------
# axon NTFF-profiling hook shim (the container's antenv lacks axon_hooks)
# ---------------------------------------------------------------------------
def _install_ntff_hook():
    if "antenv.axon_hooks" in sys.modules:
        return
    mod = types.ModuleType("antenv.axon_hooks")
    _state = {"hook": None}
    mod.set_axon_ntff_profile_hook = lambda h: _state.__setitem__("hook", h)
    mod.get_axon_ntff_profile_hook = lambda: _state["hook"]
    sys.modules["antenv.axon_hooks"] = mod
    try:
        import antenv
        antenv.axon_hooks = mod
    except ImportError:
        pass
    try:
        if "/root/.axon_site" not in sys.path:
            sys.path.insert(0, "/root/.axon_site")
        from trn_agent_boot.trn_boot import _ntff_profile_via_ctypes
        mod.set_axon_ntff_profile_hook(
            _ntff_profile_via_ctypes("/opt/axon/libaxon_pjrt.so")
        )
    except Exception:
        pass
    try:
        import concourse.bass_utils as bu
        bu.upload_artifacts = lambda tmpdir: tmpdir
    except Exception:
        pass


# ---------------------------------------------------------------------------
# custom VectorE exp2: offloads part of the softmax exp from the saturated
# ScalarE to the mostly-idle DVE.
#
#   op A (EXP2I_ANT):  out_i32 = int32((round(t) + 127) * 2^23)
#       round(t) via the +1.5*2^23 trick; the product is an exact fp32
#       integer, and the int32 OUTPUT CONVERSION writes exactly the IEEE bit
#       pattern of 2^round(t).  Reinterpreted as f32 by op B.
#   op B (EXP2F_ANT):  out = (1 + f*(c1 + c2*f)) * T,  f = t - round(t)
#       deg-2 minimax correction on [-1/2, 1/2]; max rel err 2.0e-3.
# ---------------------------------------------------------------------------
_EXP_BIG = 12582912.0          # 1.5 * 2^23
_EXP_C1 = 0.7029417939892484
_EXP_C2 = 0.23986402898768072


def _register_exp_ops():
    import numpy as np
    from concourse import dve_ops as DOPS
    from concourse.dve_spec import Spec, Src0, Src1, C0, C1, C2, One, lower
    from concourse.dve_spec import _has_src1
    from concourse.dve_uop import DveOpSpec

    if "EXP2I_ANT" in DOPS._SUB_OPCODE_FOR_NAME:
        from concourse.dve_ops import OPS
        return {op.name: op for op in OPS if op.name.startswith("EXP2")}

    def _ref_a(in0, in1, s0, s1, imm2):
        t = in0.astype(np.float32)
        i = (t + np.float32(s0)) - np.float32(s0)
        return ((i + np.float32(s1)) * np.float32(imm2)).astype(np.int32)

    y = Src0 + C0
    i = y - C0
    spec_a = Spec(body=(i + C1) * C2, reference=_ref_a)

    def _ref_b(in0, in1, s0, s1, imm2):
        t = in0.astype(np.float32)
        i = (t + np.float32(s0)) - np.float32(s0)
        f = t - i
        p = np.float32(1.0) + f * (np.float32(s1) + np.float32(imm2) * f)
        return (p * in1).astype(np.float32)

    yb = Src0 + C0
    ib = yb - C0
    fb = Src0 - ib
    spec_b = Spec(body=(One + fb * (C1 + fb * C2)) * Src1, reference=_ref_b)

    made = {}
    for name, spec in (("EXP2I_ANT", spec_a), ("EXP2F_ANT", spec_b)):
        row = DOPS._CUSTOM_DVE_ROW_BASE + len(DOPS.OPS)
        shas = {}
        for ver in ("v3", "v4"):
            dos = DveOpSpec(name=name, opcode=row, uops=lower(spec, ver=ver),
                            rd1_en=_has_src1(spec))
            shas[ver] = dos.sha(ver)
        op = DOPS.DveOp(name, spec, subdim=False, uops_sha=shas)
        DOPS._SUB_OPCODE_FOR_NAME[name] = row
        DOPS.OPS.append(op)
        DOPS.CUSTOM_DVE_SPECS[name] = spec
        made[name] = op
    return made


# kc values whose exp is SPLIT: head A's half on the DVE, head B's half on
# ScalarE.  Measured on HW: a DVE half costs 2x687ns = 1.37us (the DVE is
# half the ScalarE's element rate and needs two passes), which exceeds the
# chunk period and starves the DVE's evacuation duties - net loss, so the
# offload is disabled.  Kept for reference/experiments.
_DVE_EXP_KCS = frozenset()

# ---------------------------------------------------------------------------
# kernel build
# ---------------------------------------------------------------------------
_CACHE = {}


def _build():
    if "nc" in _CACHE:
        return _CACHE["nc"]
    exp_ops = _register_exp_ops() if _DVE_EXP_KCS else None
    nc = bacc.Bacc("TRN2", target_bir_lowering=False, debug=False,
                   num_devices=NCORES)

    xt_d = nc.dram_tensor("x_t", [C, N], BF16, kind="ExternalInput").ap()
    wqkt_d = nc.dram_tensor("w_qkt", [C, 2 * HL * D], BF16,
                            kind="ExternalInput").ap()
    wvt_d = nc.dram_tensor("w_vt", [C, HL * D], BF16,
                           kind="ExternalInput").ap()
    wprojt_d = nc.dram_tensor("w_projt", [HL * D, C], BF16,
                              kind="ExternalInput").ap()
    # bf16 partials: host sums the two half-head partials in f32 and adds
    # the bias; the quantization is well inside the rel-err budget and
    # halves the output DMA volume.
    out_d = nc.dram_tensor("out", [N, C], BF16, kind="ExternalOutput").ap()
    # reciprocal broadcast round-trip scratch (DMA cannot broadcast from SBUF)
    scratch = nc.dram_tensor("scratch", [HL, QB, QBS], F32).ap()

    with tile.TileContext(nc) as tc:
        with ExitStack() as ctx:
            pers = ctx.enter_context(tc.tile_pool(name="pers", bufs=1))
            e_pool = ctx.enter_context(tc.tile_pool(name="e", bufs=8))
            tiny = ctx.enter_context(tc.tile_pool(name="tiny", bufs=4))
            stage = ctx.enter_context(tc.tile_pool(name="stage", bufs=3))
            ps_sc = ctx.enter_context(
                tc.tile_pool(name="ps_sc", bufs=2, space="PSUM"))
            ps_out = ctx.enter_context(
                tc.tile_pool(name="ps_out", bufs=1, space="PSUM"))
            ps_mm = ctx.enter_context(
                tc.tile_pool(name="ps_mm", bufs=2, space="PSUM"))

            # persistent SBUF tensors
            xt = pers.tile([128, CCH, N], BF16)          # x^T   (c,n)
            wqkt = pers.tile([128, CCH, 2 * HL * D], BF16)
            wvt = pers.tile([128, CCH, HL * D], BF16)
            wprojt = pers.tile([128, HL * D // 128, C], BF16)
            qk = pers.tile([128, CCH, N], BF16)          # qkT rows (o,n)
            v_aug = pers.tile([128, KC, HL, D + 1], BF16)
            outT = pers.tile([128, HL * D // 128, N], BF16)

            # Input loads: one big strided DMA per logical block (each
            # dma_start trigger costs ~700ns on the issuing engine, so fewer
            # and bigger is strictly better), split across the three
            # DMA-queue engines and ordered by deadline: chunk 0 needs only
            # xt q-block 0, the k-half of wqkt, and wvt.
            xtd_v = xt_d.rearrange("(c p) n -> p c n", p=128)
            # first q-block in two halves so the v0/b chains start sooner
            nc.sync.dma_start(xt[:, 0:4, 0:512], xtd_v[:, 0:4, 0:512])
            nc.sync.dma_start(xt[:, 4:8, 0:512], xtd_v[:, 4:8, 0:512])
            wqkt_v = wqkt_d.rearrange("(c p) o -> p c o", p=128)
            # the two mo-slices chunk 0 needs land first
            nc.scalar.dma_start(wqkt[:, :, 512:640], wqkt_v[:, :, 512:640])
            nc.scalar.dma_start(wqkt[:, :, 0:128], wqkt_v[:, :, 0:128])
            nc.scalar.dma_start(wqkt[:, :, 640:1024], wqkt_v[:, :, 640:1024])
            wvt_v = wvt_d.rearrange("(c p) o -> p c o", p=128)
            nc.gpsimd.dma_start(wvt[:, 0:4, :], wvt_v[:, 0:4, :])
            nc.gpsimd.dma_start(wvt[:, 4:8, :], wvt_v[:, 4:8, :])
            nc.sync.dma_start(xt[:, :, 512:1024], xtd_v[:, :, 512:1024])
            nc.scalar.dma_start(wqkt[:, :, 128:512], wqkt_v[:, :, 128:512])
            nc.gpsimd.dma_start(
                wprojt[:, :, :], wprojt_d.rearrange("(c p) o -> p c o", p=128))
            nc.sync.dma_start(xt[:, :, 1024:1536], xtd_v[:, :, 1024:1536])
            nc.sync.dma_start(xt[:, :, 1536:2048], xtd_v[:, :, 1536:2048])

            nc.vector.memset(v_aug[:, :, :, D:D + 1], 1.0)
            ones_f32 = pers.tile([1, 64], F32, name="ones_f32")
            nc.vector.memset(ones_f32[:], 1.0)

            # PE warm-up: a dozen dependency-free matmuls on junk data keep
            # the tensor engine's HAM activity window busy during the input
            # DMA, so the real upfront chains run at 2.4GHz, not 1.2GHz.
            wrm = pers.tile([128, 512], BF16, name="wrm")
            nc.gpsimd.memset(wrm[:], 0.0)
            ps_w = ps_mm.tile([128, 512], F32, tag="mm", name="ps_w")
            for _ in range(12):
                nc.tensor.matmul(ps_w[:], lhsT=wrm[:, 0:128], rhs=wrm[:],
                                 start=True, stop=True)

            # ---- phase A: v = x @ w_v^T  (natural [n, dl] layout) ----
            # First half upfront; rest pumped just-in-time as fillers during
            # the first attention unit (chunk kc needs v[kc]).
            def v_tile(nch, as_gen):
                ps = ps_mm.tile([128, HL * D], F32, tag="mm")
                for cc in range(CCH):
                    nc.tensor.matmul(
                        ps[:],
                        lhsT=xt[:, cc, nch * 128:(nch + 1) * 128],
                        rhs=wvt[:, cc, :],
                        start=(cc == 0),
                        stop=(cc == CCH - 1),
                    )
                    if as_gen and cc < CCH - 1:
                        yield
                nc.vector.tensor_copy(
                    out=v_aug[:, nch, :, 0:D],
                    in_=ps[:].rearrange("p (h d) -> p h d", h=HL),
                )

            # ---- phase B: qkT = w_qk @ x^T ----
            def b_tile(mo, qb4, as_gen):
                ps = ps_mm.tile([128, 512], F32, tag="mm")
                for cc in range(CCH):
                    nc.tensor.matmul(
                        ps[:],
                        lhsT=wqkt[:, cc, mo * 128:(mo + 1) * 128],
                        rhs=xt[:, cc, qb4 * 512:(qb4 + 1) * 512],
                        start=(cc == 0),
                        stop=(cc == CCH - 1),
                    )
                    if as_gen and cc < CCH - 1:
                        yield
                nc.vector.tensor_copy(
                    out=qk[:, mo, qb4 * 512:(qb4 + 1) * 512], in_=ps[:])

            def run_plain(gen):
                for _ in gen:
                    pass

            # upfront: only what chunk 0 strictly needs
            done_ids = set()
            for nch in range(1):
                run_plain(v_tile(nch, False))
                done_ids.add(f"v{nch}")
            for mo, qb4 in [(4, 0), (0, 0)]:
                run_plain(b_tile(mo, qb4, False))
                done_ids.add(f"b{mo}t{qb4}")

            # Filler queue, ordered by deadline (just-in-time for chunk units).
            # CORRECTNESS: Tile's dependency tracking is emission-order based
            # (a read emitted before the producing write becomes a WAR the
            # wrong way and reads garbage silently), so every consumer must
            # ensure() its producer tiles are fully emitted first.
            # deadline order: b4t{j} needed at chunk 4j, v{kc} at chunk kc,
            # pair-1 tiles at chunk 16, ...
            fillers = []
            for fid_args in [("v1", 1), ("v2", 2), ("v3", 3), ("v4", 4),
                             ("b4t1", (4, 1)), ("v5", 5),
                             ("v6", 6), ("v7", 7), ("b4t2", (4, 2)),
                             ("v8", 8), ("v9", 9), ("v10", 10), ("v11", 11),
                             ("b4t3", (4, 3)), ("v12", 12), ("v13", 13),
                             ("v14", 14), ("v15", 15)]:
                fid, a = fid_args
                if isinstance(a, tuple):
                    fillers.append((fid, b_tile(a[0], a[1], True)))
                else:
                    fillers.append((fid, v_tile(a, True)))
            for args in [(5, 0), (1, 0), (5, 1), (5, 2), (5, 3),
                         (6, 0), (2, 0), (6, 1), (6, 2), (6, 3),
                         (7, 0), (3, 0), (7, 1), (7, 2), (7, 3),
                         (0, 1), (1, 1), (2, 1), (3, 1),
                         (0, 2), (1, 2), (2, 2), (3, 2),
                         (0, 3), (1, 3), (2, 3), (3, 3)]:
                mo, qb4 = args
                fillers.append((f"b{mo}t{qb4}", b_tile(mo, qb4, True)))

            # Low-priority queue: projection tiles have no early deadline, so
            # they are rationed at <=1 matmul per chunk to land in the
            # late-kernel ScalarE-gated slack instead of extending the
            # PE-bound stretches.
            lp_fillers = []

            def _step_front(q):
                fid, gen = q[0]
                try:
                    next(gen)
                    return 1
                except StopIteration:
                    done_ids.add(fid)
                    q.pop(0)
                    return 0

            def pump(n):
                emitted = 0
                while emitted < n and fillers:
                    emitted += _step_front(fillers)
                if not fillers and lp_fillers and emitted < n:
                    emitted += _step_front(lp_fillers)

            def ensure(fid):
                # Jump the queue: run ONLY the matching generator to
                # completion.  v-tiles and QKV b-tiles are emission-
                # independent, and draining the whole queue front here
                # produced 6-8us bursts that blocked the next chunk's QK in
                # the PE FIFO (measured as the dominant exp-chain gaps).
                if fid in done_ids:
                    return
                for k, (qid, gen) in enumerate(fillers):
                    if qid == fid:
                        for _ in gen:
                            pass
                        done_ids.add(fid)
                        fillers.pop(k)
                        return

            # output projection tiles, pumped as fillers once outT(qb) exists
            def proj_tile(nch, ob):
                ps = ps_mm.tile([128, 512], F32, tag="mm")
                ncc = HL * D // 128
                for cc in range(ncc):
                    nc.tensor.matmul(
                        ps[:],
                        lhsT=outT[:, cc, nch * 128:(nch + 1) * 128],
                        rhs=wprojt[:, cc, ob * 512:(ob + 1) * 512],
                        start=(cc == 0),
                        stop=(cc == ncc - 1),
                    )
                    if cc < ncc - 1:
                        yield
                st = stage.tile([128, 512], BF16, tag="st")
                nc.vector.tensor_copy(out=st[:], in_=ps[:])
                nc.sync.dma_start(
                    out_d[nch * 128:(nch + 1) * 128,
                          ob * 512:(ob + 1) * 512], st[:])

            # ---- attention chunk stream, software-pipelined ----
            chunks = [(qb, p, kc)
                      for qb in range(QB) for p in range(4) for kc in range(KC)]
            NCH = len(chunks)

            def emit_qk(i, sc):
                qb, p, kc = chunks[i]
                q0 = qb * QBS
                ensure(f"b{p}t{qb}")
                ensure(f"b{4 + p}t{kc // 4}")
                for par in range(2):     # head 2p (par=0), 2p+1 (par=1)
                    pp = par * 64
                    nc.tensor.matmul(
                        sc[:, par * 512:(par + 1) * 512],
                        lhsT=qk[pp:pp + 64, 4 + p, kc * 128:(kc + 1) * 128],
                        rhs=qk[pp:pp + 64, p, q0:q0 + QBS],
                        start=True,
                        stop=True,
                    )

            sc_tiles = [None] * NCH
            e_tiles = [None] * NCH
            acc_cur = [None]

            # exp chunk j: 2^t (q pre-scaled by 0.125*log2e on host).  For
            # split chunks the DVE computes head A's half while ScalarE does
            # head B's half - each fits inside one chunk period.
            def emit_exp(j):
                _, _, kcj = chunks[j]
                sc = sc_tiles[j]
                e_t = e_pool.tile([128, 1024], BF16, tag="e")
                e_tiles[j] = e_t
                if kcj in _DVE_EXP_KCS:
                    ti = e_pool.tile([128, 512], I32, tag="ti")
                    nc.vector._custom_dve(
                        exp_ops["EXP2I_ANT"], out=ti[:], in0=sc[:, 0:512],
                        s0=_EXP_BIG, s1=127.0, imm2=8388608.0)
                    nc.vector._custom_dve(
                        exp_ops["EXP2F_ANT"], out=e_t[:, 0:512],
                        in0=sc[:, 0:512], in1=ti[:].bitcast(F32),
                        s0=_EXP_BIG, s1=_EXP_C1, imm2=_EXP_C2)
                    nc.scalar.activation(
                        out=e_t[:, 512:1024], in_=sc[:, 512:1024],
                        func=mybir.ActivationFunctionType.Exp, scale=LN2)
                else:
                    nc.scalar.activation(
                        out=e_t[:], in_=sc[:],
                        func=mybir.ActivationFunctionType.Exp, scale=LN2)
                sc_tiles[j] = None

            sc0 = ps_sc.tile([128, 1024], F32, tag="sc")
            sc_tiles[0] = sc0
            emit_qk(0, sc0)
            emit_exp(0)

            for i in range(NCH):
                qb, p, kc = chunks[i]
                q0 = qb * QBS
                # pipelined: QK and exp of chunk i+1 are emitted before the
                # AV of chunk i, so e(i) has a full chunk period of slack
                if i + 1 < NCH:
                    sc_n = ps_sc.tile([128, 1024], F32, tag="sc")
                    sc_tiles[i + 1] = sc_n
                    emit_qk(i + 1, sc_n)
                    emit_exp(i + 1)
                e_t = e_tiles[i]
                # AV accumulation (M=65 full-tiled; row 64 = sumexp)
                ensure(f"v{kc}")
                if kc == 0:
                    acc_cur[0] = ps_out.tile([65, 1024], F32, tag="acc",
                                             name="acc")
                acc = acc_cur[0]
                st0, sp0 = (kc == 0), (kc == KC - 1)
                nc.tensor.matmul(
                    acc[:, 0:512], lhsT=v_aug[:, kc, 2 * p, :],
                    rhs=e_t[:, 0:512], start=st0, stop=sp0)
                nc.tensor.matmul(
                    acc[:, 512:1024], lhsT=v_aug[:, kc, 2 * p + 1, :],
                    rhs=e_t[:, 512:1024], start=st0, stop=sp0)
                pump(2)
                # unit epilogue: evacuate acc fast (one f32 copy per head;
                # row 64 is the sumexp), then deferred normalization
                if sp0:
                    nufs = []
                    for par in range(2):
                        c0 = par * 512
                        nuf = tiny.tile([65, QBS], F32, tag="nu")
                        nc.vector.tensor_copy(out=nuf[:],
                                              in_=acc[:, c0:c0 + 512])
                        nufs.append(nuf)
                    for par in range(2):
                        h = 2 * p + par
                        nuf = nufs[par]
                        g = tiny.tile([128, QBS // 128], F32, tag="g")
                        nc.sync.dma_start(g[:], nuf[64:65, :])
                        r = tiny.tile([128, QBS // 128], F32, tag="r")
                        nc.vector.reciprocal(out=r[:], in_=g[:])
                        nc.sync.dma_start(scratch[h, qb:qb + 1, :], r[:])
                        bc = tiny.tile([64, QBS], F32, tag="bc")
                        # NOTE: issue the broadcast from GpSimd (SWDGE), not
                        # nc.sync. Tile's HWDGE DMA->DMA cross-queue wait
                        # placement is unsound for this pair (the broadcast's
                        # wait landed on the wrong queue semaphore and raced
                        # the scatter on cold runs); an engine-issued DMA gets
                        # a sound engine-level wait on the scatter's
                        # completion semaphore.
                        nc.gpsimd.dma_start(
                            bc[:],
                            scratch[h, qb:qb + 1, :].to_broadcast((64, QBS)))
                        pp = par * 64
                        nc.vector.tensor_mul(
                            outT[pp:pp + 64, p, q0:q0 + QBS],
                            nuf[0:64, :], bc[:])
                # projection fillers for q-block qb-1, appended one unit into
                # q-block qb so the first proj matmul never waits on the
                # just-written outT (its bc-DMA chain has ~2-3us latency)
                if sp0 and p == 0 and qb > 0:
                    for j in range(4):
                        for ob in range(2):
                            lp_fillers.append(
                                (f"proj{qb - 1}_{j}_{ob}",
                                 proj_tile((qb - 1) * 4 + j, ob)))

            # keep the PE's HAM window warm while the last unit's
            # normalization chain (DMA round-trips) completes, so the tail
            # projection runs at full clock
            for k in range(14):
                ps_t = ps_mm.tile([128, 512], F32, tag="mm", name="ps_t")
                nc.tensor.matmul(ps_t[:], lhsT=wrm[:, 0:128], rhs=wrm[:],
                                 start=True, stop=True)
            for j in range(4):
                for ob in range(2):
                    lp_fillers.append(
                        (f"proj3_{j}_{ob}", proj_tile(3 * 4 + j, ob)))
            while fillers or lp_fillers:
                pump(1000)
                while lp_fillers:
                    _step_front(lp_fillers)

    nc.compile()
    _CACHE["nc"] = nc
    return nc


# ---------------------------------------------------------------------------
# host wrapper
# ---------------------------------------------------------------------------
def kernel(x, w_qkv, w_proj, b_proj):
    _install_ntff_hook()
    from concourse.bass_utils import run_bass_kernel_spmd

    x = np.asarray(x, dtype=np.float32)
    w_qkv = np.asarray(w_qkv, dtype=np.float32)
    w_proj = np.asarray(w_proj, dtype=np.float32)
    b_proj = np.asarray(b_proj, dtype=np.float32)
    B = x.shape[0]

    nc = _build()

    def bf(a):
        return np.ascontiguousarray(a).astype(ml_dtypes.bfloat16)

    in_maps = []
    for core in range(NCORES):
        b, hg = core // 2, core % 2
        sl = slice(hg * HL * D, (hg + 1) * HL * D)
        w_q = w_qkv[0 * C:1 * C][sl] * (SCALE * LOG2E)   # fold softmax scale
        w_k = w_qkv[1 * C:2 * C][sl]
        w_v = w_qkv[2 * C:3 * C][sl]
        w_qk_t = np.concatenate([w_q, w_k], axis=0).T   # [C, 1024]
        in_maps.append({
            "x_t": bf(x[b].T),                  # [C, N]
            "w_qkt": bf(w_qk_t),
            "w_vt": bf(w_v.T),                  # [C, 512]
            "w_projt": bf(w_proj[:, sl].T),     # [512, C]
        })

    trace = bool(int(os.environ.get("KERNEL_TRACE", "0")))
    res = run_bass_kernel_spmd(nc, in_maps, core_ids=list(range(NCORES)),
                               trace=trace)
    kernel.last_results = res

    out = np.empty((B, N, C), dtype=np.float32)
    for b in range(B):
        out[b] = (res.results[2 * b]["out"].astype(np.float32)
                  + res.results[2 * b + 1]["out"].astype(np.float32))
        out[b] += b_proj
    return out


if __name__ == "__main__":
    t0 = time.time()
    _build()
    print(f"build+compile: {time.time()-t0:.1f}s")


# revision 63
# speedup vs baseline: 1.0047x; 1.0047x over previous
"""Multi-head attention kernel for Trainium2, SPMD over 8 NeuronCores.

Problem: B=4, N=2048, C=1024, 16 heads, head_dim=64 (fp32 reference).

Sharding: core = (batch b, head-group hg) with b = core//2, hg = core%2.
Each core computes attention for its 8 heads of its batch and a PARTIAL
projection output [N, C]; the host sums the two partials per batch and adds
the bias. No on-chip collectives needed.

Per-core dataflow (all matmul inputs bf16, f32 PSUM accumulation):
  qkT[o,n]  = w_qk_local @ x^T          (o = [q heads | k heads], 1024 rows)
  v[n,dl]   = x @ w_v_local^T           (natural layout, augmented with ones col)
  S_T[k,q]  = kT_h^T q (per head)       -> 2^(S_T) on ScalarE -> E bf16
              (q pre-scaled by 0.125*log2(e) on host, ACT scale=ln2)
  numT[d,q] = v_aug^T E  (M=65: row 64 = sumexp)   accumulated over k chunks
  outT[d,q] = numT[0:64] * (1/sumexp)   (recip via DMA gather + DVE, DMA bcast)
  partial[n,o] = outT^T @ w_proj_local^T chunks    -> DMA out (f32)

Scheduling (the perf-critical part; measured on HW):
  * Consecutive full-array (auto-tiled) matmuls chain at ~226 ns (their
    LDWEIGHTS hides via the background weight buffer); explicitly tiled
    matmuls pay ~+90 ns per category transition.  So AV stays M=65 (full
    tiling) and the only tiled matmuls are the QK row-pair (which runs
    internally concurrent at +4 ns).
  * The chunk loop is software-pipelined: QK(i+1) is emitted BEFORE AV(i),
    so the PE FIFO never idles while ScalarE computes exp(i) - the per-chunk
    serial QK->exp->AV chain was the baseline's main stall.
  * QKV projection (phase B) and the output projection run as "filler"
    matmuls pumped into the PE slack of the exp-gated chunk stream.
"""
import os
import sys
import types
import time
import numpy as np
import ml_dtypes
from contextlib import ExitStack

import concourse.bass as bass
import concourse.tile as tile
from concourse import bacc, mybir

BF16 = mybir.dt.bfloat16
F32 = mybir.dt.float32
I32 = mybir.dt.int32

N = 2048          # sequence length
C = 1024          # model dim
HL = 8            # heads per core
D = 64            # head dim
SCALE = D ** -0.5
NCORES = 8
LN2 = float(np.log(2.0))
LOG2E = float(np.log2(np.e))

KC = N // 128     # 16 k-chunks per head
QB = 4            # q blocks of 512
QBS = N // QB     # 512
CCH = C // 128    # 8 contraction chunks


# ---------------------------------------------------------------------------
# axon NTFF-profiling hook shim (the container's antenv lacks axon_hooks)
# ---------------------------------------------------------------------------
def _install_ntff_hook():
    if "antenv.axon_hooks" in sys.modules:
        return
    mod = types.ModuleType("antenv.axon_hooks")
    _state = {"hook": None}
    mod.set_axon_ntff_profile_hook = lambda h: _state.__setitem__("hook", h)
    mod.get_axon_ntff_profile_hook = lambda: _state["hook"]
    sys.modules["antenv.axon_hooks"] = mod
    try:
        import antenv
        antenv.axon_hooks = mod
    except ImportError:
        pass
    try:
        if "/root/.axon_site" not in sys.path:
            sys.path.insert(0, "/root/.axon_site")
        from trn_agent_boot.trn_boot import _ntff_profile_via_ctypes
        mod.set_axon_ntff_profile_hook(
            _ntff_profile_via_ctypes("/opt/axon/libaxon_pjrt.so")
        )
    except Exception:
        pass
    try:
        import concourse.bass_utils as bu
        bu.upload_artifacts = lambda tmpdir: tmpdir
    except Exception:
        pass


# ---------------------------------------------------------------------------
# custom VectorE exp2: offloads part of the softmax exp from the saturated
# ScalarE to the mostly-idle DVE.
#
#   op A (EXP2I_ANT):  out_i32 = int32((round(t) + 127) * 2^23)
#       round(t) via the +1.5*2^23 trick; the product is an exact fp32
#       integer, and the int32 OUTPUT CONVERSION writes exactly the IEEE bit
#       pattern of 2^round(t).  Reinterpreted as f32 by op B.
#   op B (EXP2F_ANT):  out = (1 + f*(c1 + c2*f)) * T,  f = t - round(t)
#       deg-2 minimax correction on [-1/2, 1/2]; max rel err 2.0e-3.
# ---------------------------------------------------------------------------
_EXP_BIG = 12582912.0          # 1.5 * 2^23
_EXP_C1 = 0.7029417939892484
_EXP_C2 = 0.23986402898768072


def _register_exp_ops():
    import numpy as np
    from concourse import dve_ops as DOPS
    from concourse.dve_spec import Spec, Src0, Src1, C0, C1, C2, One, lower
    from concourse.dve_spec import _has_src1
    from concourse.dve_uop import DveOpSpec

    if "EXP2I_ANT" in DOPS._SUB_OPCODE_FOR_NAME:
        from concourse.dve_ops import OPS
        return {op.name: op for op in OPS if op.name.startswith("EXP2")}

    def _ref_a(in0, in1, s0, s1, imm2):
        t = in0.astype(np.float32)
        i = (t + np.float32(s0)) - np.float32(s0)
        return ((i + np.float32(s1)) * np.float32(imm2)).astype(np.int32)

    y = Src0 + C0
    i = y - C0
    spec_a = Spec(body=(i + C1) * C2, reference=_ref_a)

    def _ref_b(in0, in1, s0, s1, imm2):
        t = in0.astype(np.float32)
        i = (t + np.float32(s0)) - np.float32(s0)
        f = t - i
        p = np.float32(1.0) + f * (np.float32(s1) + np.float32(imm2) * f)
        return (p * in1).astype(np.float32)

    yb = Src0 + C0
    ib = yb - C0
    fb = Src0 - ib
    spec_b = Spec(body=(One + fb * (C1 + fb * C2)) * Src1, reference=_ref_b)

    made = {}
    for name, spec in (("EXP2I_ANT", spec_a), ("EXP2F_ANT", spec_b)):
        row = DOPS._CUSTOM_DVE_ROW_BASE + len(DOPS.OPS)
        shas = {}
        for ver in ("v3", "v4"):
            dos = DveOpSpec(name=name, opcode=row, uops=lower(spec, ver=ver),
                            rd1_en=_has_src1(spec))
            shas[ver] = dos.sha(ver)
        op = DOPS.DveOp(name, spec, subdim=False, uops_sha=shas)
        DOPS._SUB_OPCODE_FOR_NAME[name] = row
        DOPS.OPS.append(op)
        DOPS.CUSTOM_DVE_SPECS[name] = spec
        made[name] = op
    return made


# kc values whose exp is SPLIT: head A's half on the DVE, head B's half on
# ScalarE.  Measured on HW: a DVE half costs 2x687ns = 1.37us (the DVE is
# half the ScalarE's element rate and needs two passes), which exceeds the
# chunk period and starves the DVE's evacuation duties - net loss, so the
# offload is disabled.  Kept for reference/experiments.
_DVE_EXP_KCS = frozenset()

# ---------------------------------------------------------------------------
# kernel build
# ---------------------------------------------------------------------------
_CACHE = {}


def _build():
    if "nc" in _CACHE:
        return _CACHE["nc"]
    exp_ops = _register_exp_ops() if _DVE_EXP_KCS else None
    nc = bacc.Bacc("TRN2", target_bir_lowering=False, debug=False,
                   num_devices=NCORES)

    xt_d = nc.dram_tensor("x_t", [C, N], BF16, kind="ExternalInput").ap()
    wqkt_d = nc.dram_tensor("w_qkt", [C, 2 * HL * D], BF16,
                            kind="ExternalInput").ap()
    wvt_d = nc.dram_tensor("w_vt", [C, HL * D], BF16,
                           kind="ExternalInput").ap()
    wprojt_d = nc.dram_tensor("w_projt", [HL * D, C], BF16,
                              kind="ExternalInput").ap()
    # bf16 partials: host sums the two half-head partials in f32 and adds
    # the bias; the quantization is well inside the rel-err budget and
    # halves the output DMA volume.
    out_d = nc.dram_tensor("out", [N, C], BF16, kind="ExternalOutput").ap()
    # reciprocal broadcast round-trip scratch (DMA cannot broadcast from SBUF)
    scratch = nc.dram_tensor("scratch", [HL, QB, QBS], F32).ap()

    with tile.TileContext(nc) as tc:
        with ExitStack() as ctx:
            pers = ctx.enter_context(tc.tile_pool(name="pers", bufs=1))
            e_pool = ctx.enter_context(tc.tile_pool(name="e", bufs=8))
            tiny = ctx.enter_context(tc.tile_pool(name="tiny", bufs=4))
            stage = ctx.enter_context(tc.tile_pool(name="stage", bufs=3))
            ps_sc = ctx.enter_context(
                tc.tile_pool(name="ps_sc", bufs=2, space="PSUM"))
            ps_out = ctx.enter_context(
                tc.tile_pool(name="ps_out", bufs=1, space="PSUM"))
            ps_mm = ctx.enter_context(
                tc.tile_pool(name="ps_mm", bufs=2, space="PSUM"))

            # persistent SBUF tensors
            xt = pers.tile([128, CCH, N], BF16)          # x^T   (c,n)
            wqkt = pers.tile([128, CCH, 2 * HL * D], BF16)
            wvt = pers.tile([128, CCH, HL * D], BF16)
            wprojt = pers.tile([128, HL * D // 128, C], BF16)
            qk = pers.tile([128, CCH, N], BF16)          # qkT rows (o,n)
            v_aug = pers.tile([128, KC, HL, D + 1], BF16)
            outT = pers.tile([128, HL * D // 128, N], BF16)

            # Input loads: one big strided DMA per logical block (each
            # dma_start trigger costs ~700ns on the issuing engine, so fewer
            # and bigger is strictly better), split across the three
            # DMA-queue engines and ordered by deadline: chunk 0 needs only
            # xt q-block 0, the k-half of wqkt, and wvt.
            xtd_v = xt_d.rearrange("(c p) n -> p c n", p=128)
            # first q-block in two halves so the v0/b chains start sooner
            nc.sync.dma_start(xt[:, 0:4, 0:512], xtd_v[:, 0:4, 0:512])
            nc.sync.dma_start(xt[:, 4:8, 0:512], xtd_v[:, 4:8, 0:512])
            wqkt_v = wqkt_d.rearrange("(c p) o -> p c o", p=128)
            # the two mo-slices chunk 0 needs land first
            nc.scalar.dma_start(wqkt[:, :, 512:640], wqkt_v[:, :, 512:640])
            nc.scalar.dma_start(wqkt[:, :, 0:128], wqkt_v[:, :, 0:128])
            nc.scalar.dma_start(wqkt[:, :, 640:1024], wqkt_v[:, :, 640:1024])
            wvt_v = wvt_d.rearrange("(c p) o -> p c o", p=128)
            nc.gpsimd.dma_start(wvt[:, 0:4, :], wvt_v[:, 0:4, :])
            nc.gpsimd.dma_start(wvt[:, 4:8, :], wvt_v[:, 4:8, :])
            nc.sync.dma_start(xt[:, :, 512:1024], xtd_v[:, :, 512:1024])
            nc.scalar.dma_start(wqkt[:, :, 128:512], wqkt_v[:, :, 128:512])
            nc.gpsimd.dma_start(
                wprojt[:, :, :], wprojt_d.rearrange("(c p) o -> p c o", p=128))
            nc.sync.dma_start(xt[:, :, 1024:1536], xtd_v[:, :, 1024:1536])
            nc.sync.dma_start(xt[:, :, 1536:2048], xtd_v[:, :, 1536:2048])

            nc.vector.memset(v_aug[:, :, :, D:D + 1], 1.0)
            ones_f32 = pers.tile([1, 64], F32, name="ones_f32")
            nc.vector.memset(ones_f32[:], 1.0)

            # PE warm-up: a dozen dependency-free matmuls on junk data keep
            # the tensor engine's HAM activity window busy during the input
            # DMA, so the real upfront chains run at 2.4GHz, not 1.2GHz.
            wrm = pers.tile([128, 512], BF16, name="wrm")
            nc.gpsimd.memset(wrm[:], 0.0)
            ps_w = ps_mm.tile([128, 512], F32, tag="mm", name="ps_w")
            for _ in range(12):
                nc.tensor.matmul(ps_w[:], lhsT=wrm[:, 0:128], rhs=wrm[:],
                                 start=True, stop=True)

            # ---- phase A: v = x @ w_v^T  (natural [n, dl] layout) ----
            # First half upfront; rest pumped just-in-time as fillers during
            # the first attention unit (chunk kc needs v[kc]).
            def v_tile(nch, as_gen):
                ps = ps_mm.tile([128, HL * D], F32, tag="mm")
                for cc in range(CCH):
                    nc.tensor.matmul(
                        ps[:],
                        lhsT=xt[:, cc, nch * 128:(nch + 1) * 128],
                        rhs=wvt[:, cc, :],
                        start=(cc == 0),
                        stop=(cc == CCH - 1),
                    )
                    if as_gen and cc < CCH - 1:
                        yield
                nc.vector.tensor_copy(
                    out=v_aug[:, nch, :, 0:D],
                    in_=ps[:].rearrange("p (h d) -> p h d", h=HL),
                )

            # ---- phase B: qkT = w_qk @ x^T ----
            def b_tile(mo, qb4, as_gen):
                ps = ps_mm.tile([128, 512], F32, tag="mm")
                for cc in range(CCH):
                    nc.tensor.matmul(
                        ps[:],
                        lhsT=wqkt[:, cc, mo * 128:(mo + 1) * 128],
                        rhs=xt[:, cc, qb4 * 512:(qb4 + 1) * 512],
                        start=(cc == 0),
                        stop=(cc == CCH - 1),
                    )
                    if as_gen and cc < CCH - 1:
                        yield
                nc.vector.tensor_copy(
                    out=qk[:, mo, qb4 * 512:(qb4 + 1) * 512], in_=ps[:])

            def run_plain(gen):
                for _ in gen:
                    pass

            # upfront: only what chunk 0 strictly needs
            done_ids = set()
            for nch in range(1):
                run_plain(v_tile(nch, False))
                done_ids.add(f"v{nch}")
            for mo, qb4 in [(4, 0), (0, 0)]:
                run_plain(b_tile(mo, qb4, False))
                done_ids.add(f"b{mo}t{qb4}")

            # Filler queue, ordered by deadline (just-in-time for chunk units).
            # CORRECTNESS: Tile's dependency tracking is emission-order based
            # (a read emitted before the producing write becomes a WAR the
            # wrong way and reads garbage silently), so every consumer must
            # ensure() its producer tiles are fully emitted first.
            # deadline order: b4t{j} needed at chunk 4j, v{kc} at chunk kc,
            # pair-1 tiles at chunk 16, ...
            fillers = []
            for fid_args in [("v1", 1), ("v2", 2), ("v3", 3), ("v4", 4),
                             ("b4t1", (4, 1)), ("v5", 5),
                             ("v6", 6), ("v7", 7), ("b4t2", (4, 2)),
                             ("v8", 8), ("v9", 9), ("v10", 10), ("v11", 11),
                             ("b4t3", (4, 3)), ("v12", 12), ("v13", 13),
                             ("v14", 14), ("v15", 15)]:
                fid, a = fid_args
                if isinstance(a, tuple):
                    fillers.append((fid, b_tile(a[0], a[1], True)))
                else:
                    fillers.append((fid, v_tile(a, True)))
            for args in [(5, 0), (1, 0), (5, 1), (5, 2), (5, 3),
                         (6, 0), (2, 0), (6, 1), (6, 2), (6, 3),
                         (7, 0), (3, 0), (7, 1), (7, 2), (7, 3),
                         (0, 1), (1, 1), (2, 1), (3, 1),
                         (0, 2), (1, 2), (2, 2), (3, 2),
                         (0, 3), (1, 3), (2, 3), (3, 3)]:
                mo, qb4 = args
                fillers.append((f"b{mo}t{qb4}", b_tile(mo, qb4, True)))

            # Low-priority queue: projection tiles have no early deadline, so
            # they are rationed at <=1 matmul per chunk to land in the
            # late-kernel ScalarE-gated slack instead of extending the
            # PE-bound stretches.
            lp_fillers = []

            def _step_front(q):
                fid, gen = q[0]
                try:
                    next(gen)
                    return 1
                except StopIteration:
                    done_ids.add(fid)
                    q.pop(0)
                    return 0

            def pump(n):
                emitted = 0
                while emitted < n and fillers:
                    emitted += _step_front(fillers)
                if not fillers and lp_fillers and emitted < n:
                    emitted += _step_front(lp_fillers)

            def ensure(fid):
                while fid not in done_ids and fillers:
                    _step_front(fillers)

            # output projection tiles, pumped as fillers once outT(qb) exists
            def proj_tile(nch, ob):
                ps = ps_mm.tile([128, 512], F32, tag="mm")
                ncc = HL * D // 128
                for cc in range(ncc):
                    nc.tensor.matmul(
                        ps[:],
                        lhsT=outT[:, cc, nch * 128:(nch + 1) * 128],
                        rhs=wprojt[:, cc, ob * 512:(ob + 1) * 512],
                        start=(cc == 0),
                        stop=(cc == ncc - 1),
                    )
                    if cc < ncc - 1:
                        yield
                st = stage.tile([128, 512], BF16, tag="st")
                nc.vector.tensor_copy(out=st[:], in_=ps[:])
                nc.sync.dma_start(
                    out_d[nch * 128:(nch + 1) * 128,
                          ob * 512:(ob + 1) * 512], st[:])

            # ---- attention chunk stream, software-pipelined ----
            chunks = [(qb, p, kc)
                      for qb in range(QB) for p in range(4) for kc in range(KC)]
            NCH = len(chunks)

            def emit_qk(i, sc):
                qb, p, kc = chunks[i]
                q0 = qb * QBS
                ensure(f"b{p}t{qb}")
                ensure(f"b{4 + p}t{kc // 4}")
                for par in range(2):     # head 2p (par=0), 2p+1 (par=1)
                    pp = par * 64
                    nc.tensor.matmul(
                        sc[:, par * 512:(par + 1) * 512],
                        lhsT=qk[pp:pp + 64, 4 + p, kc * 128:(kc + 1) * 128],
                        rhs=qk[pp:pp + 64, p, q0:q0 + QBS],
                        start=True,
                        stop=True,
                    )

            sc_tiles = [None] * NCH
            e_tiles = [None] * NCH
            acc_cur = [None]

            # exp chunk j: 2^t (q pre-scaled by 0.125*log2e on host).  For
            # split chunks the DVE computes head A's half while ScalarE does
            # head B's half - each fits inside one chunk period.
            def emit_exp(j):
                _, _, kcj = chunks[j]
                sc = sc_tiles[j]
                e_t = e_pool.tile([128, 1024], BF16, tag="e")
                e_tiles[j] = e_t
                if kcj in _DVE_EXP_KCS:
                    ti = e_pool.tile([128, 512], I32, tag="ti")
                    nc.vector._custom_dve(
                        exp_ops["EXP2I_ANT"], out=ti[:], in0=sc[:, 0:512],
                        s0=_EXP_BIG, s1=127.0, imm2=8388608.0)
                    nc.vector._custom_dve(
                        exp_ops["EXP2F_ANT"], out=e_t[:, 0:512],
                        in0=sc[:, 0:512], in1=ti[:].bitcast(F32),
                        s0=_EXP_BIG, s1=_EXP_C1, imm2=_EXP_C2)
                    nc.scalar.activation(
                        out=e_t[:, 512:1024], in_=sc[:, 512:1024],
                        func=mybir.ActivationFunctionType.Exp, scale=LN2)
                else:
                    nc.scalar.activation(
                        out=e_t[:], in_=sc[:],
                        func=mybir.ActivationFunctionType.Exp, scale=LN2)
                sc_tiles[j] = None

            sc0 = ps_sc.tile([128, 1024], F32, tag="sc")
            sc_tiles[0] = sc0
            emit_qk(0, sc0)
            emit_exp(0)

            for i in range(NCH):
                qb, p, kc = chunks[i]
                q0 = qb * QBS
                # pipelined: QK and exp of chunk i+1 are emitted before the
                # AV of chunk i, so e(i) has a full chunk period of slack
                if i + 1 < NCH:
                    sc_n = ps_sc.tile([128, 1024], F32, tag="sc")
                    sc_tiles[i + 1] = sc_n
                    emit_qk(i + 1, sc_n)
                    emit_exp(i + 1)
                e_t = e_tiles[i]
                # AV accumulation (M=65 full-tiled; row 64 = sumexp)
                ensure(f"v{kc}")
                if kc == 0:
                    acc_cur[0] = ps_out.tile([65, 1024], F32, tag="acc",
                                             name="acc")
                acc = acc_cur[0]
                st0, sp0 = (kc == 0), (kc == KC - 1)
                nc.tensor.matmul(
                    acc[:, 0:512], lhsT=v_aug[:, kc, 2 * p, :],
                    rhs=e_t[:, 0:512], start=st0, stop=sp0)
                nc.tensor.matmul(
                    acc[:, 512:1024], lhsT=v_aug[:, kc, 2 * p + 1, :],
                    rhs=e_t[:, 512:1024], start=st0, stop=sp0)
                pump(2)
                # unit epilogue: evacuate acc fast (one f32 copy per head;
                # row 64 is the sumexp), then deferred normalization
                if sp0:
                    nufs = []
                    for par in range(2):
                        c0 = par * 512
                        nuf = tiny.tile([65, QBS], F32, tag="nu")
                        nc.vector.tensor_copy(out=nuf[:],
                                              in_=acc[:, c0:c0 + 512])
                        nufs.append(nuf)
                    for par in range(2):
                        h = 2 * p + par
                        nuf = nufs[par]
                        g = tiny.tile([128, QBS // 128], F32, tag="g")
                        nc.sync.dma_start(g[:], nuf[64:65, :])
                        r = tiny.tile([128, QBS // 128], F32, tag="r")
                        nc.vector.reciprocal(out=r[:], in_=g[:])
                        nc.sync.dma_start(scratch[h, qb:qb + 1, :], r[:])
                        bc = tiny.tile([64, QBS], F32, tag="bc")
                        # NOTE: issue the broadcast from GpSimd (SWDGE), not
                        # nc.sync. Tile's HWDGE DMA->DMA cross-queue wait
                        # placement is unsound for this pair (the broadcast's
                        # wait landed on the wrong queue semaphore and raced
                        # the scatter on cold runs); an engine-issued DMA gets
                        # a sound engine-level wait on the scatter's
                        # completion semaphore.
                        nc.gpsimd.dma_start(
                            bc[:],
                            scratch[h, qb:qb + 1, :].to_broadcast((64, QBS)))
                        pp = par * 64
                        nc.vector.tensor_mul(
                            outT[pp:pp + 64, p, q0:q0 + QBS],
                            nuf[0:64, :], bc[:])
                # projection fillers for q-block qb-1, appended one unit into
                # q-block qb so the first proj matmul never waits on the
                # just-written outT (its bc-DMA chain has ~2-3us latency)
                if sp0 and p == 0 and qb > 0:
                    for j in range(4):
                        for ob in range(2):
                            lp_fillers.append(
                                (f"proj{qb - 1}_{j}_{ob}",
                                 proj_tile((qb - 1) * 4 + j, ob)))

            # keep the PE's HAM window warm while the last unit's
            # normalization chain (DMA round-trips) completes, so the tail
            # projection runs at full clock
            for k in range(14):
                ps_t = ps_mm.tile([128, 512], F32, tag="mm", name="ps_t")
                nc.tensor.matmul(ps_t[:], lhsT=wrm[:, 0:128], rhs=wrm[:],
                                 start=True, stop=True)
            for j in range(4):
                for ob in range(2):
                    lp_fillers.append(
                        (f"proj3_{j}_{ob}", proj_tile(3 * 4 + j, ob)))
            while fillers or lp_fillers:
                pump(1000)
                while lp_fillers:
                    _step_front(lp_fillers)

    nc.compile()
    _CACHE["nc"] = nc
    return nc


# ---------------------------------------------------------------------------
# host wrapper
# ---------------------------------------------------------------------------
def kernel(x, w_qkv, w_proj, b_proj):
    _install_ntff_hook()
    from concourse.bass_utils import run_bass_kernel_spmd

    x = np.asarray(x, dtype=np.float32)
    w_qkv = np.asarray(w_qkv, dtype=np.float32)
    w_proj = np.asarray(w_proj, dtype=np.float32)
    b_proj = np.asarray(b_proj, dtype=np.float32)
    B = x.shape[0]

    nc = _build()

    def bf(a):
        return np.ascontiguousarray(a).astype(ml_dtypes.bfloat16)

    in_maps = []
    for core in range(NCORES):
        b, hg = core // 2, core % 2
        sl = slice(hg * HL * D, (hg + 1) * HL * D)
        w_q = w_qkv[0 * C:1 * C][sl] * (SCALE * LOG2E)   # fold softmax scale
        w_k = w_qkv[1 * C:2 * C][sl]
        w_v = w_qkv[2 * C:3 * C][sl]
        w_qk_t = np.concatenate([w_q, w_k], axis=0).T   # [C, 1024]
        in_maps.append({
            "x_t": bf(x[b].T),                  # [C, N]
            "w_qkt": bf(w_qk_t),
            "w_vt": bf(w_v.T),                  # [C, 512]
            "w_projt": bf(w_proj[:, sl].T),     # [512, C]
        })

    trace = bool(int(os.environ.get("KERNEL_TRACE", "0")))
    res = run_bass_kernel_spmd(nc, in_maps, core_ids=list(range(NCORES)),
                               trace=trace)
    kernel.last_results = res

    out = np.empty((B, N, C), dtype=np.float32)
    for b in range(B):
        out[b] = (res.results[2 * b]["out"].astype(np.float32)
                  + res.results[2 * b + 1]["out"].astype(np.float32))
        out[b] += b_proj
    return out


if __name__ == "__main__":
    t0 = time.time()
    _build()
    print(f"build+compile: {time.time()-t0:.1f}s")
